# revision 6
# baseline (speedup 1.0000x reference)
"""GAT (3-layer, DGL-style) forward pass on 8 Trainium2 NeuronCores.

Strategy (dst-node sharded, graph-parallel):
  - Nodes are partitioned into 8 contiguous shards (dst ownership); edges are
    grouped by owner(dst) and 128-node dst tile. Edge slots within a tile are
    p-major over a [128, NCH] slot grid.
  - Per layer, each core computes feat_aug = h_shard @ [W@M | W@Ar] for its
    own shard. M is a per-head basis change whose first column is al, so
    el = feat'[h*32] comes back with the gather for free. The bf16 feat'
    table is exchanged via two half-shard AllGathers (overlapping the
    second half of the producing edge phase).
  - Edge phase per dst tile: one indirect (hardware-DGE) gather per tile
    fetches all src rows with int32 table-row indices; ex =
    exp(leaky_relu(el+er)) is built as max(exp(e), exp(0.2e)); er is
    expanded edge-wise with a host-precomputed transposed one-hot matmul;
    one-hot matmuls accumulate sum(ex*feat') and sum(ex) per dst node in
    PSUM (softmax max-subtraction cancels; logits are O(1)).
  - Epilogue (layers 1-2): normalize, transpose, un-prime via an Minv
    matmul, residual + ELU in the transposed layout; the result directly
    feeds the next layer's dense matmul and stays in SBUF for residuals.
    Layer 3: projected residual and the classifier fused as in a dense
    epilogue; output rows DMA'd per tile.

All core-dependent information lives in per-core input tensors, so every core
runs an identical SPMD program.
"""

import sys

import numpy as np

for _p in ("/opt/trn_rl_repo",):
    if _p not in sys.path:
        sys.path.insert(0, _p)

import ml_dtypes

BF16 = ml_dtypes.bfloat16

P = 128
NEG_SLOPE = 0.2
HID = 32
N_CLS = 40
N_CORES = 8

_PROGRAM_CACHE = {}


# ----------------------------------------------------------------------------
# Host-side preparation (index manipulation / sharding only)
# ----------------------------------------------------------------------------

def _head_basis(al):
    """Per-head basis M (first column = al_h) and its exact inverse."""
    H, D = al.shape
    M = np.zeros((H * D, H * D), np.float64)
    Minv = np.zeros((H * D, H * D), np.float64)
    for h in range(H):
        a = al[h].astype(np.float64)
        nrm2 = a @ a
        assert nrm2 > 1e-12
        Q, _ = np.linalg.qr(np.concatenate([a[:, None], np.eye(D)], axis=1))
        Mh = np.concatenate([a[:, None], Q[:, 1:D]], axis=1)
        Mih = np.concatenate([(a / nrm2)[None, :], Q[:, 1:D].T], axis=0)
        M[h * D:(h + 1) * D, h * D:(h + 1) * D] = Mh
        Minv[h * D:(h + 1) * D, h * D:(h + 1) * D] = Mih
    return M, Minv


def _ar_proj(W, ar):
    """[in, H] f64: W @ Ar where Ar embeds ar per head."""
    H, D = ar.shape
    A = np.zeros((H * D, H), np.float64)
    A[np.arange(H * D), np.arange(H * D) // D] = ar.reshape(-1).astype(np.float64)
    return W.astype(np.float64) @ A


def _prepare(x, src, dst, n_cores=N_CORES):
    n_nodes = x.shape[0]
    assert n_nodes % n_cores == 0
    shard = n_nodes // n_cores
    shard_pad = ((shard + P - 1) // P) * P
    T = shard_pad // P
    assert T % 2 == 0
    HB = T // 2                  # collective half boundary (tiles)
    BR = shard_pad // 2          # rows per half-block
    # table rows are half-block-major: [half0: core0..7 | half1: core0..7]
    zrow = n_cores * BR          # half1, core0, first row — any valid row is
    # fine for padding slots (they are masked by a zero one-hot); this one is
    # a real row so the gathered bits are always finite.

    owner = dst // shard
    lt = dst - owner * shard
    tt = lt // P
    dofv = (lt % P).astype(np.int64)
    src_c = (src // shard).astype(np.int64)
    src_l = (src % shard).astype(np.int64)
    kh = src_l // BR
    srow = (kh * (n_cores * BR) + src_c * BR + (src_l - kh * BR)).astype(np.int32)

    group = owner * T + tt
    n_groups = n_cores * T
    counts = np.bincount(group, minlength=n_groups).reshape(n_cores, T)
    ni_t = counts.max(axis=0)
    nch_t = np.maximum((ni_t + P - 1) // P, 1).astype(np.int64)
    NCHMAX = int(nch_t.max())
    mw_t = nch_t + (nch_t + 1) // 2
    MWMAX = int(mw_t.max())

    order = np.argsort(group, kind="stable")
    g_sorted = group[order]
    starts = np.zeros(n_groups + 1, np.int64)
    np.cumsum(np.bincount(group, minlength=n_groups), out=starts[1:])
    pos = np.arange(len(order)) - starts[g_sorted]

    oc = g_sorted // T
    tc = g_sorted % T
    nch_e = nch_t[tc]
    pe = pos // nch_e
    ce = pos % nch_e

    gidx = np.full((n_cores, T, P, NCHMAX), zrow, np.int32)
    gidx[oc, tc, pe, ce] = srow[order]
    neg1 = np.float32(-1).astype(BF16).view(np.int16)
    dof16 = np.full((n_cores, T, P, NCHMAX), neg1, np.int16)
    dof_bf = dofv.astype(np.float32).astype(BF16).view(np.int16)
    dof16[oc, tc, pe, ce] = dof_bf[order]

    # meta32[t]: int32 cols [0, NCH) = gather row idx; int16 cols
    # [2*NCH, 3*NCH) = bf16-encoded dst offsets.
    meta32 = np.zeros((n_cores, T, P, MWMAX), np.int32)
    meta16 = meta32.view(np.int16).reshape(n_cores, T, P, 2 * MWMAX)
    for t in range(T):
        nch = int(nch_t[t])
        meta32[:, t, :, :nch] = gidx[:, t, :, :nch]
        meta16[:, t, :, 2 * nch:3 * nch] = dof16[:, t, :, :nch]

    # transposed one-hot for er expansion: oht[i, c*128+p] = (dof[p,c] == i)
    dof_int = np.full((n_cores, T, P, NCHMAX), -1, np.int16)
    dof_int[oc, tc, pe, ce] = dofv[order].astype(np.int16)
    i_ar = np.arange(P, dtype=np.int16)
    oht = (dof_int[:, :, None, :, :] == i_ar[None, None, :, None, None])
    # [C, T, i, p, c] -> [C, T, i, c, p]
    oht = oht.transpose(0, 1, 2, 4, 3).astype(BF16)
    oht = np.ascontiguousarray(oht.reshape(n_cores, T * P, NCHMAX * P))

    xT_per_core = []
    for c in range(n_cores):
        xs = x[c * shard:(c + 1) * shard].astype(np.float32)
        if shard_pad != shard:
            xs = np.concatenate(
                [xs, np.zeros((shard_pad - shard, xs.shape[1]), np.float32)], 0)
        xT_per_core.append(np.ascontiguousarray(xs.T))

    return dict(
        shard=shard, shard_pad=shard_pad, T=T, HB=HB, BR=BR,
        NCHMAX=NCHMAX, MWMAX=MWMAX,
        nch_t=nch_t.tolist(), mw_t=mw_t.tolist(),
        meta_per_core=[np.ascontiguousarray(
            meta32[c].reshape(T * P, MWMAX)) for c in range(n_cores)],
        oht_per_core=[np.ascontiguousarray(oht[c]) for c in range(n_cores)],
        xT_per_core=xT_per_core,
    )


# ----------------------------------------------------------------------------
# Device program
# ----------------------------------------------------------------------------

def _build_program(n_cores, plan, has_bias):
    from concourse import bacc, bass, tile
    import concourse.mybir as mybir
    from concourse.masks import make_identity

    dt = mybir.dt
    f32, bf16, i16, i32 = dt.float32, dt.bfloat16, dt.int16, dt.int32
    Alu = mybir.AluOpType
    Act = mybir.ActivationFunctionType

    shard, SP, T = plan["shard"], plan["shard_pad"], plan["T"]
    HB, BR = plan["HB"], plan["BR"]
    NCHMAX, MWMAX = plan["NCHMAX"], plan["MWMAX"]
    nch_t = plan["nch_t"]
    rg = [list(range(n_cores))]
    WCOL = [P, P, 64]            # table row widths per layer

    nc = bacc.Bacc("TRN2", target_bir_lowering=False, debug=False,
                   num_devices=n_cores)

    xT = nc.dram_tensor("xT", [P, SP], f32, kind="ExternalInput")
    meta_d = nc.dram_tensor("meta", [T * P, MWMAX], i32, kind="ExternalInput")
    ohtd = nc.dram_tensor("ohtd", [T * P, NCHMAX * P], bf16, kind="ExternalInput")
    waug1 = nc.dram_tensor("waug1", [P, 132], f32, kind="ExternalInput")
    waug2 = nc.dram_tensor("waug2", [P, 132], f32, kind="ExternalInput")
    waug3 = nc.dram_tensor("waug3", [P, 64], f32, kind="ExternalInput")
    minv1d = nc.dram_tensor("minv1", [P, P], f32, kind="ExternalInput")
    minv2d = nc.dram_tensor("minv2", [P, P], f32, kind="ExternalInput")
    res3w = nc.dram_tensor("res3w", [P, HID], f32, kind="ExternalInput")
    wfc = nc.dram_tensor("wfc", [HID, N_CLS], f32, kind="ExternalInput")
    bias_d = [None] * 2
    bias_shapes = [(P, HID), (P, N_CLS)]
    for i, hb in enumerate(has_bias):
        if hb:
            bias_d[i] = nc.dram_tensor(f"bias{i}", list(bias_shapes[i]), f32,
                                       kind="ExternalInput")
    out_e = nc.dram_tensor("out", [shard, N_CLS], f32, kind="ExternalOutput")

    agA = [nc.dram_tensor(f"agA{l}", [BR, WCOL[l]], bf16, kind="Internal")
           for l in range(3)]
    agB = [nc.dram_tensor(f"agB{l}", [BR, WCOL[l]], bf16, kind="Internal")
           for l in range(3)]
    tables = [nc.dram_tensor(f"table{l}", [2 * n_cores * BR, WCOL[l]], bf16,
                             kind="Internal", addr_space="Shared")
              for l in range(3)]

    with tile.TileContext(nc) as tc:
        with (
            tc.tile_pool(name="const", bufs=1) as cpool,
            tc.tile_pool(name="big", bufs=1) as bigpool,
            tc.tile_pool(name="gth", bufs=3) as gpool,
            tc.tile_pool(name="oht", bufs=3) as opool,
            tc.tile_pool(name="work", bufs=3) as wpool,
            tc.tile_pool(name="wsm", bufs=3) as spool,
            tc.tile_pool(name="pagg", bufs=2, space="PSUM") as p_agg,
            tc.tile_pool(name="ptr", bufs=2, space="PSUM") as p_tr,
            tc.tile_pool(name="pdn", bufs=2, space="PSUM") as p_dn,
            tc.tile_pool(name="per", bufs=1, space="PSUM") as p_er,
            tc.tile_pool(name="prs", bufs=1, space="PSUM") as p_rs,
        ):
            ident = cpool.tile([P, P], f32)
            make_identity(nc, ident[:])
            iota_i = cpool.tile([P, P], i32)
            nc.gpsimd.iota(iota_i[:], pattern=[[1, P]], base=0, channel_multiplier=0)
            iota_bf = cpool.tile([P, P], bf16)
            nc.vector.tensor_copy(iota_bf[:], iota_i[:])

            w1_sb = cpool.tile([P, 132], f32)
            nc.sync.dma_start(w1_sb[:], waug1[:, :])
            w2_sb = cpool.tile([P, 132], f32)
            nc.sync.dma_start(w2_sb[:], waug2[:, :])
            w3_sb = cpool.tile([P, 64], f32)
            nc.sync.dma_start(w3_sb[:], waug3[:, :])
            minv_sb = [cpool.tile([P, P], f32, name=f"minv{i}_sb") for i in range(2)]
            nc.sync.dma_start(minv_sb[0][:], minv1d[:, :])
            nc.sync.dma_start(minv_sb[1][:], minv2d[:, :])
            res3_sb = cpool.tile([P, HID], f32)
            nc.sync.dma_start(res3_sb[:], res3w[:, :])
            wfc_sb = cpool.tile([HID, N_CLS], f32)
            nc.sync.dma_start(wfc_sb[:], wfc[:, :])
            bias_sb = [None] * 2
            for i, d in enumerate(bias_d):
                if d is not None:
                    bias_sb[i] = cpool.tile(list(bias_shapes[i]), f32)
                    nc.sync.dma_start(bias_sb[i][:], d[:, :])

            h1T = bigpool.tile([P, SP], f32)
            h2T = bigpool.tile([P, SP], f32)
            er_sb = [bigpool.tile([P, T, 4], bf16, name=f"er{i}_sb") for i in range(3)]

            def bcast_mid(ap, n):
                return bass.AP(ap.tensor, ap.offset, [ap.ap[0], [0, n], ap.ap[1]])

            def agin_write(li, t, fsb):
                """DMA a [P, W] bf16 dense tile into the right half-block."""
                if t < HB:
                    nc.sync.dma_start(agA[li][t * P:(t + 1) * P, :], fsb)
                else:
                    r = (t - HB) * P
                    nc.sync.dma_start(agB[li][r:r + P, :], fsb)

            def dense_tile(li, t, lhsT_ap):
                """feat' for layer li (0-based) tile t: write agin + er_sb."""
                w_sb = (w1_sb, w2_sb, w3_sb)[li]
                ncols = (132, 132, 64)[li]
                eroff, H = ((128, 4), (128, 4), (33, 1))[li]
                ps = p_dn.tile([P, ncols], f32, tag="ps_dense")
                nc.tensor.matmul(ps[:], lhsT=lhsT_ap, rhs=w_sb[:], start=True,
                                 stop=True)
                fsb = wpool.tile([P, P], bf16, tag="fsb")
                W = WCOL[li]
                nc.scalar.activation(fsb[:, :W], ps[:, :W], Act.Copy)
                agin_write(li, t, fsb[:, :W])
                nc.scalar.activation(er_sb[li][:, t, :H], ps[:, eroff:eroff + H],
                                     Act.Copy)

            def cc_half(li, half):
                ag = (agA, agB)[half][li]
                o0 = half * n_cores * BR
                nc.gpsimd.collective_compute(
                    "AllGather", Alu.bypass, replica_groups=rg,
                    ins=[ag[:, :]],
                    outs=[tables[li][o0:o0 + n_cores * BR, :]])

            # layer-1 dense from xT; fire half-collectives as halves complete
            for t in range(T):
                lh = wpool.tile([P, P], f32, tag="xt_t")
                nc.sync.dma_start(lh[:], xT[:, t * P:(t + 1) * P])
                dense_tile(0, t, lh[:])
                if t == HB - 1:
                    cc_half(0, 0)
            cc_half(0, 1)

            def edge_phase(layer):  # 1-based
                li = layer - 1
                H = 4 if layer < 3 else 1
                FE = H * HID
                W = WCOL[li]
                table = tables[li]
                act = layer < 3
                oh_eng = nc.vector  # is_equal is not in the Pool ISA
                for t in range(T):
                    r0 = t * P
                    NCH = nch_t[t]
                    MW = NCH + (NCH + 1) // 2
                    meta = spool.tile([P, MWMAX], i32, tag="meta")
                    nc.sync.dma_start(meta[:, :MW], meta_d[r0:r0 + P, :MW])
                    ohts = opool.tile([P, NCHMAX, P], bf16, tag="oht")
                    nc.sync.dma_start(ohts[:, :NCH, :],
                                      ohtd[r0:r0 + P, :NCH * P])
                    if act:
                        gsb = gpool.tile([P, NCHMAX, P], bf16, tag="gsb")
                    else:
                        gsb = gpool.tile([P, NCHMAX, 64], bf16, tag="gsb3")
                    for c in range(NCH):
                        nc.gpsimd.indirect_dma_start(
                            out=gsb[:, c, :W],
                            out_offset=None,
                            in_=table[:, :],
                            in_offset=bass.IndirectOffsetOnAxis(
                                ap=meta[:, c:c + 1], axis=0),
                        )
                    dofb = meta[:].bitcast(bf16)[:, 2 * NCH:3 * NCH]
                    oh = wpool.tile([P, NCHMAX, P], bf16, tag="oh")
                    oh_eng.tensor_tensor(
                        out=oh[:, :NCH, :], in0=bcast_mid(iota_bf[:, :], NCH),
                        in1=dofb.to_broadcast([P, NCH, P]), op=Alu.is_equal)
                    # er expansion via host transposed one-hot
                    pse = p_er.tile([P, NCHMAX * 4], f32, tag="ps_er")
                    for c in range(NCH):
                        nc.tensor.matmul(
                            pse[:, c * H:(c + 1) * H],
                            lhsT=ohts[:, c, :], rhs=er_sb[li][:, t, :H],
                            start=True, stop=True)
                    # e = el + er ; el is the first column of each head block
                    el_ap = (gsb[:, :NCH, 0:P:HID] if act
                             else gsb[:, :NCH, HID:HID + 1])
                    esb = spool.tile([P, NCHMAX, H], f32, tag="e")
                    nc.vector.tensor_tensor(
                        out=esb[:, :NCH, :], in0=el_ap,
                        in1=pse[:, :NCH * H].rearrange("p (c h) -> p c h", h=H),
                        op=Alu.add)
                    # ex = exp(leaky_relu(e)) = max(exp(e), exp(0.2*e))
                    ex1 = spool.tile([P, NCHMAX, H], f32, tag="ex1")
                    nc.scalar.activation(ex1[:, :NCH, :], esb[:, :NCH, :], Act.Exp)
                    ex2 = spool.tile([P, NCHMAX, H], f32, tag="ex2")
                    nc.scalar.activation(ex2[:, :NCH, :], esb[:, :NCH, :], Act.Exp,
                                         scale=NEG_SLOPE)
                    ex = spool.tile([P, NCHMAX, H], f32, tag="ex")
                    nc.vector.tensor_tensor(out=ex[:, :NCH, :], in0=ex1[:, :NCH, :],
                                            in1=ex2[:, :NCH, :], op=Alu.max)
                    # g = [feat'*ex | ex]
                    g = wpool.tile([P, NCHMAX, FE + H], bf16, tag="g")
                    nc.vector.tensor_tensor(
                        out=g[:, :NCH, 0:FE].rearrange("p c (h d) -> p c h d", h=H),
                        in0=gsb[:, :NCH, 0:FE].rearrange("p c (h d) -> p c h d", h=H),
                        in1=ex[:, :NCH, :].to_broadcast([P, NCH, H, HID]),
                        op=Alu.mult)
                    nc.scalar.activation(g[:, :NCH, FE:FE + H], ex[:, :NCH, :],
                                         Act.Copy)
                    # aggregate
                    psa = p_agg.tile([P, FE + H], f32, tag="ps_agg")
                    for c in range(NCH):
                        nc.tensor.matmul(psa[:], lhsT=oh[:, c, :], rhs=g[:, c, :],
                                         start=(c == 0), stop=(c == NCH - 1))
                    ssb = spool.tile([P, H], f32, tag="s")
                    nc.vector.tensor_scalar_max(ssb[:], psa[:, FE:FE + H], 1e-30)
                    rec = spool.tile([P, H], f32, tag="rec")
                    nc.vector.reciprocal(rec[:], ssb[:])
                    if act:
                        osb = wpool.tile([P, FE], f32, tag="osb")
                        nc.vector.tensor_tensor(
                            out=osb[:].rearrange("p (h d) -> p h d", h=H),
                            in0=psa[:, 0:FE].rearrange("p (h d) -> p h d", h=H),
                            in1=rec[:].to_broadcast([P, H, HID]), op=Alu.mult)
                        # transpose, un-prime, residual, ELU — all [feat, dst]
                        pst = p_tr.tile([P, P], f32, tag="ps_t")
                        nc.tensor.transpose(pst[:], osb[:], ident[:])
                        tsb = wpool.tile([P, P], f32, tag="tsb")
                        nc.scalar.activation(tsb[:], pst[:], Act.Copy)
                        psu = p_tr.tile([P, P], f32, tag="ps_t")
                        nc.tensor.matmul(psu[:], lhsT=minv_sb[li][:], rhs=tsb[:],
                                         start=True, stop=True)
                        if layer == 2:
                            srcb = wpool.tile([P, P], f32, tag="srcb")
                            nc.vector.tensor_tensor(out=srcb[:], in0=psu[:],
                                                    in1=h1T[:, r0:r0 + P],
                                                    op=Alu.add)
                            src = srcb[:]
                        else:
                            src = psu[:]
                        rl = wpool.tile([P, P], f32, tag="rl")
                        nc.scalar.activation(rl[:], src, Act.Relu)
                        e3 = wpool.tile([P, P], f32, tag="e3")
                        nc.scalar.activation(e3[:], src, Act.Exp)
                        e4 = wpool.tile([P, P], f32, tag="e4")
                        nc.vector.tensor_scalar(e4[:], e3[:], 1.0, -1.0,
                                                Alu.min, Alu.add)
                        hdst = (h1T if layer == 1 else h2T)[:, r0:r0 + P]
                        nc.vector.tensor_tensor(out=hdst, in0=rl[:], in1=e4[:],
                                                op=Alu.add)
                        dense_tile(li + 1, t, hdst)
                        if t == HB - 1:
                            cc_half(li + 1, 0)
                        elif t == T - 1:
                            cc_half(li + 1, 1)
                    else:
                        psr = p_rs.tile([P, HID], f32, tag="ps_res")
                        nc.tensor.matmul(psr[:], lhsT=h2T[:, r0:r0 + P],
                                         rhs=res3_sb[:], start=True, stop=True)
                        osb = wpool.tile([P, HID], f32, tag="osb3")
                        nc.vector.tensor_tensor(
                            out=osb[:].rearrange("p (h d) -> p h d", h=1),
                            in0=psa[:, 0:HID].rearrange("p (h d) -> p h d", h=1),
                            in1=rec[:].to_broadcast([P, 1, HID]), op=Alu.mult)
                        nc.vector.tensor_tensor(out=osb[:], in0=osb[:],
                                                in1=psr[:], op=Alu.add)
                        if bias_sb[0] is not None:
                            nc.vector.tensor_tensor(out=osb[:], in0=osb[:],
                                                    in1=bias_sb[0][:, :],
                                                    op=Alu.add)
                        pst = p_tr.tile([P, P], f32, tag="ps_t")
                        nc.tensor.transpose(pst[:HID, :], osb[:], ident[:])
                        hts = spool.tile([HID, P], f32, tag="h3t")
                        nc.scalar.activation(hts[:], pst[:HID, :], Act.Copy)
                        psf = p_dn.tile([P, N_CLS], f32, tag="ps_dense")
                        nc.tensor.matmul(psf[:], lhsT=hts[:], rhs=wfc_sb[:],
                                         start=True, stop=True)
                        ofc = spool.tile([P, N_CLS], f32, tag="ofc")
                        nc.scalar.activation(ofc[:], psf[:], Act.Copy)
                        if bias_sb[1] is not None:
                            nc.vector.tensor_tensor(out=ofc[:], in0=ofc[:],
                                                    in1=bias_sb[1][:, :],
                                                    op=Alu.add)
                        rows = min(shard - r0, P)
                        if rows > 0:
                            nc.sync.dma_start(out_e[r0:r0 + rows, :],
                                              ofc[:rows, :])

            edge_phase(1)
            edge_phase(2)
            edge_phase(3)

    nc.compile()
    return nc


def _get_program(n_cores, plan, has_bias):
    key = (n_cores, plan["shard"], plan["NCHMAX"], plan["MWMAX"],
           tuple(plan["nch_t"]), has_bias)
    if key not in _PROGRAM_CACHE:
        _PROGRAM_CACHE[key] = _build_program(n_cores, plan, has_bias)
    return _PROGRAM_CACHE[key]


def _make_in_maps(prep, inputs, has_bias, n_cores=N_CORES):
    W1 = np.asarray(inputs["W1"], np.float32).astype(np.float64)
    W2 = np.asarray(inputs["W2"], np.float32).astype(np.float64)
    W3 = np.asarray(inputs["W3"], np.float32).astype(np.float64)
    al1 = np.asarray(inputs["al1"], np.float32)
    al2 = np.asarray(inputs["al2"], np.float32)
    al3 = np.asarray(inputs["al3"], np.float32)
    M1, Mi1 = _head_basis(al1)
    M2, Mi2 = _head_basis(al2)
    waug1 = np.concatenate([W1 @ M1, _ar_proj(W1, np.asarray(inputs["ar1"]))],
                           axis=1).astype(np.float32)
    waug2 = np.concatenate([W2 @ M2, _ar_proj(W2, np.asarray(inputs["ar2"]))],
                           axis=1).astype(np.float32)
    al3p = _ar_proj(W3, al3)
    ar3p = _ar_proj(W3, np.asarray(inputs["ar3"]))
    waug3 = np.concatenate(
        [W3, al3p, ar3p, np.zeros((P, 64 - HID - 2), np.float64)],
        axis=1).astype(np.float32)

    biases = []
    shapes = [(P, HID), (P, N_CLS)]
    for i, nm in enumerate(("b3", "bfc")):
        b = np.asarray(inputs[nm], np.float32).reshape(1, -1)
        biases.append(np.ascontiguousarray(np.broadcast_to(b, shapes[i])))
    in_maps = []
    for c in range(n_cores):
        m = dict(
            xT=prep["xT_per_core"][c],
            meta=prep["meta_per_core"][c],
            ohtd=prep["oht_per_core"][c],
            waug1=waug1, waug2=waug2, waug3=waug3,
            minv1=Mi1.astype(np.float32), minv2=Mi2.astype(np.float32),
            res3w=np.asarray(inputs["res3"], np.float32),
            wfc=np.asarray(inputs["Wfc"], np.float32),
        )
        for i, hb in enumerate(has_bias):
            if hb:
                m[f"bias{i}"] = biases[i]
        in_maps.append(m)
    return in_maps


def run_gat(inputs, n_cores=N_CORES, trace=False):
    """Builds (cached), runs on hardware, returns (output, BassKernelResults)."""
    from concourse import bass_utils

    x, src, dst = inputs["x"], inputs["src"], inputs["dst"]
    prep = _prepare(x, src, dst, n_cores)
    assert not np.any(np.asarray(inputs["b1"])), "b1 must be zero (pad rows)"
    assert not np.any(np.asarray(inputs["b2"])), "b2 must be zero (pad rows)"
    has_bias = tuple(
        bool(np.any(np.asarray(inputs[nm]))) for nm in ("b3", "bfc"))
    nc = _get_program(n_cores, prep, has_bias)
    in_maps = _make_in_maps(prep, inputs, has_bias, n_cores)
    res = bass_utils.run_bass_kernel_spmd(
        nc, in_maps, core_ids=list(range(n_cores)), trace=trace)
    out = np.concatenate([r["out"] for r in res.results], axis=0)
    return out[: x.shape[0]].astype(np.float32), res


def kernel(**inputs):
    out, _ = run_gat(inputs)
    return out


# revision 8
# speedup vs baseline: 1.9999x; 1.9999x over previous
"""GAT (3-layer, DGL-style) forward pass on 8 Trainium2 NeuronCores.

Strategy (dst-node sharded, graph-parallel):
  - Nodes are partitioned into 8 contiguous shards (dst ownership); edges are
    grouped by owner(dst), dst-tile pair, table quarter (int16 gather range)
    and tile within the pair. Slots are chunk-major in a per-pair buffer, so
    one dma_gather per (pair, quarter) covers both tiles of the pair.
  - Per layer, each core computes feat_aug = h_shard @ [W@M | W@Ar] for its
    own shard. M is a per-head basis change whose first column is al, so
    el = feat'[h*32] comes back with the gather for free. The bf16 feat'
    table is exchanged via two half-shard AllGathers (the first overlaps the
    second half of the producing edge phase).
  - Edge phase per pair: 4 SWDGE gathers (queue per quarter) fetch src rows;
    ex = exp(leaky_relu(el+er)) is built as max(exp(e), exp(0.2e)); er is
    expanded edge-wise with a host-precomputed transposed one-hot matmul;
    one-hot matmuls accumulate sum(ex*feat') and sum(ex) per dst node in
    PSUM (softmax max-subtraction cancels; logits are O(1)). Gather tails
    are left stale (finite) and masked by the one-hots; the gather pool is
    zeroed once at startup.
  - Epilogue (layers 1-2): normalize, transpose, un-prime via an Minv
    matmul, residual + ELU in the transposed layout; the result directly
    feeds the next layer's dense matmul and stays in SBUF for residuals.
    Layer 3: projected residual and the classifier fused; rows DMA'd out.

All core-dependent information lives in per-core input tensors, so every core
runs an identical SPMD program.
"""

import sys

import numpy as np

for _p in ("/opt/trn_rl_repo",):
    if _p not in sys.path:
        sys.path.insert(0, _p)

import ml_dtypes

BF16 = ml_dtypes.bfloat16

P = 128
NEG_SLOPE = 0.2
HID = 32
N_CLS = 40
N_CORES = 8
NQ = 4
NI_HW_MAX = 1024

_PROGRAM_CACHE = {}


# ----------------------------------------------------------------------------
# Host-side preparation (index manipulation / sharding only)
# ----------------------------------------------------------------------------

def _head_basis(al):
    """Per-head basis M (first column = al_h) and its exact inverse."""
    H, D = al.shape
    M = np.zeros((H * D, H * D), np.float64)
    Minv = np.zeros((H * D, H * D), np.float64)
    for h in range(H):
        a = al[h].astype(np.float64)
        nrm2 = a @ a
        assert nrm2 > 1e-12
        Q, _ = np.linalg.qr(np.concatenate([a[:, None], np.eye(D)], axis=1))
        Mh = np.concatenate([a[:, None], Q[:, 1:D]], axis=1)
        Mih = np.concatenate([(a / nrm2)[None, :], Q[:, 1:D].T], axis=0)
        M[h * D:(h + 1) * D, h * D:(h + 1) * D] = Mh
        Minv[h * D:(h + 1) * D, h * D:(h + 1) * D] = Mih
    return M, Minv


def _ar_proj(W, ar):
    H, D = ar.shape
    A = np.zeros((H * D, H), np.float64)
    A[np.arange(H * D), np.arange(H * D) // D] = ar.reshape(-1).astype(np.float64)
    return W.astype(np.float64) @ A


def _prepare(x, src, dst, n_cores=N_CORES):
    n_nodes = x.shape[0]
    assert n_nodes % n_cores == 0
    shard = n_nodes // n_cores
    shard_pad = ((shard + P - 1) // P) * P
    T = shard_pad // P
    assert T % 2 == 0
    NJ = T // 2                 # tile pairs
    HB = T // 2                 # collective half boundary (tiles)
    BR = shard_pad // 2         # rows per half-block
    nrows = 2 * n_cores * BR
    assert nrows % NQ == 0
    QROWS = nrows // NQ
    assert QROWS <= 32767

    owner = dst // shard
    lt = dst - owner * shard
    tt = lt // P
    dofv = (lt % P).astype(np.int64)
    src_c = (src // shard).astype(np.int64)
    src_l = (src % shard).astype(np.int64)
    kh = src_l // BR
    srow = (kh * (n_cores * BR) + src_c * BR + (src_l - kh * BR)).astype(np.int64)
    quarter = srow // QROWS
    qidx = (srow % QROWS).astype(np.int16)

    jj = tt // 2
    mm = tt % 2
    group = ((owner * NJ + jj) * NQ + quarter) * 2 + mm
    n_groups = n_cores * NJ * NQ * 2
    counts = np.bincount(group, minlength=n_groups).reshape(n_cores, NJ, NQ, 2)
    ni = counts.max(axis=0)                      # [NJ, NQ, 2]
    ch = (ni + P - 1) // P                       # chunks
    # chunk base per (j, q, m) in the pair buffer; quarters outer, tile inner
    chq = ch.sum(axis=2)                         # [NJ, NQ]
    qbase = np.concatenate([np.zeros((NJ, 1), np.int64),
                            np.cumsum(chq, axis=1)[:, :NQ - 1]], axis=1)
    B = qbase[:, :, None] + np.concatenate(
        [np.zeros((NJ, NQ, 1), np.int64), ch[:, :, :1]], axis=2)   # [NJ,NQ,2]
    nchp = chq.sum(axis=1)                       # chunks per pair
    NCHP = int(nchp.max())

    # gather segments per (j, q): merged [num_idxs, chunk_base] or split
    # streams are padded to full 128-chunks (pad index 0 gathers a real,
    # finite row that the zero one-hot columns mask out), so every slot of
    # every chunk is always written — no stale-data reasoning needed.
    segs = [[None] * NQ for _ in range(NJ)]
    icol = np.zeros((NJ, NQ, 2), np.int64)       # stream 16-col base per seg
    icw_j = np.zeros(NJ, np.int64)
    for j in range(NJ):
        pos16 = 0
        for q in range(NQ):
            c0, c1 = int(ch[j, q, 0]), int(ch[j, q, 1])
            merged_n = (c0 + c1) * P
            if c0 and c1 and merged_n <= NI_HW_MAX:
                segs[j][q] = [(int(B[j, q, 0]), pos16, merged_n)]
                icol[j, q, 0] = pos16 * 16
                icol[j, q, 1] = pos16 * 16 + c0 * P
                pos16 += merged_n // 16
            else:
                ss = []
                for m, cm in ((0, c0), (1, c1)):
                    icol[j, q, m] = pos16 * 16
                    if cm:
                        ss.append((int(B[j, q, m]), pos16, cm * P))
                        pos16 += cm * P // 16
                segs[j][q] = ss
        icw_j[j] = pos16
    ICW = int(icw_j.max())
    MW = ICW + NCHP              # i16 meta columns: [gidx wrap | dof]

    order = np.argsort(group, kind="stable")
    g_sorted = group[order]
    starts = np.zeros(n_groups + 1, np.int64)
    np.cumsum(np.bincount(group, minlength=n_groups), out=starts[1:])
    pos = np.arange(len(order)) - starts[g_sorted]

    oc = g_sorted // (NJ * NQ * 2)
    rem = g_sorted % (NJ * NQ * 2)
    jc = rem // (NQ * 2)
    qc = (rem // 2) % NQ
    mc = rem % 2

    # slot in pair buffer (chunk-major)
    slot = B[jc, qc, mc] * P + pos
    pe = slot % P
    ce = slot // P
    dof16 = np.full((n_cores, NJ, P, NCHP),
                    np.float32(-1).astype(BF16).view(np.int16), np.int16)
    dof16[oc, jc, pe, ce] = dofv[order].astype(np.float32).astype(BF16).view(
        np.int16)
    dof_int = np.full((n_cores, NJ, P, NCHP), -1, np.int16)
    dof_int[oc, jc, pe, ce] = dofv[order].astype(np.int16)

    # gather index stream (16-wrapped)
    gpos = icol[jc, qc, mc] + pos
    gidx_all = np.zeros((n_cores, NJ, ICW * 16), np.int16)
    gidx_all[oc, jc, gpos] = qidx[order]
    gidx_w = gidx_all.reshape(n_cores, NJ, ICW, 16).transpose(0, 1, 3, 2)
    gidx_w = np.broadcast_to(gidx_w[:, :, None, :, :],
                             (n_cores, NJ, 8, 16, ICW))
    gidx_w = np.ascontiguousarray(gidx_w).reshape(n_cores, NJ * P, ICW)

    meta = np.zeros((n_cores, NJ, P, MW), np.int16)
    meta[:, :, :, :ICW] = gidx_w.reshape(n_cores, NJ, P, ICW)
    meta[:, :, :, ICW:] = dof16

    # transposed one-hot for er expansion
    i_ar = np.arange(P, dtype=np.int16)
    oht = (dof_int[:, :, None, :, :] == i_ar[None, None, :, None, None])
    oht = oht.transpose(0, 1, 2, 4, 3).astype(BF16)      # [C, NJ, i, c, p]
    oht = np.ascontiguousarray(oht.reshape(n_cores, NJ * P, NCHP * P))

    # chunk -> (tile-in-pair, is_first, is_last) per pair, in buffer order
    chunk_plan = []
    for j in range(NJ):
        plan = []
        for q in range(NQ):
            for m in range(2):
                for k in range(int(ch[j, q, m])):
                    plan.append((m, int(B[j, q, m]) + k))
        bym = {0: [], 1: []}
        for m, cidx in plan:
            bym[m].append(cidx)
        chunk_plan.append((bym[0], bym[1]))
    seg_plan = [[segs[j][q] for q in range(NQ)] for j in range(NJ)]

    xT_per_core = []
    for c in range(n_cores):
        xs = x[c * shard:(c + 1) * shard].astype(np.float32)
        if shard_pad != shard:
            xs = np.concatenate(
                [xs, np.zeros((shard_pad - shard, xs.shape[1]), np.float32)], 0)
        xT_per_core.append(np.ascontiguousarray(xs.T))

    return dict(
        shard=shard, shard_pad=shard_pad, T=T, NJ=NJ, HB=HB, BR=BR,
        QROWS=QROWS, NCHP=NCHP, ICW=ICW, MW=MW,
        nchp=nchp.tolist(),
        chunk_plan=chunk_plan, seg_plan=seg_plan,
        meta_per_core=[np.ascontiguousarray(
            meta[c].reshape(NJ * P, MW)) for c in range(n_cores)],
        oht_per_core=[np.ascontiguousarray(oht[c]) for c in range(n_cores)],
        xT_per_core=xT_per_core,
    )


# ----------------------------------------------------------------------------
# Device program
# ----------------------------------------------------------------------------

def _build_program(n_cores, plan, has_bias):
    from concourse import bacc, bass, tile
    import concourse.mybir as mybir
    from concourse.masks import make_identity

    dt = mybir.dt
    f32, bf16, i16, i32 = dt.float32, dt.bfloat16, dt.int16, dt.int32
    Alu = mybir.AluOpType
    Act = mybir.ActivationFunctionType

    shard, SP, T = plan["shard"], plan["shard_pad"], plan["T"]
    NJ, HB, BR, QROWS = plan["NJ"], plan["HB"], plan["BR"], plan["QROWS"]
    NCHP, ICW, MW = plan["NCHP"], plan["ICW"], plan["MW"]
    nchp = plan["nchp"]
    chunk_plan, seg_plan = plan["chunk_plan"], plan["seg_plan"]
    rg = [list(range(n_cores))]

    nc = bacc.Bacc("TRN2", target_bir_lowering=False, debug=False,
                   num_devices=n_cores, num_swdge_queues=4)

    xT = nc.dram_tensor("xT", [P, SP], f32, kind="ExternalInput")
    meta_d = nc.dram_tensor("meta", [NJ * P, MW], i16, kind="ExternalInput")
    ohtd = nc.dram_tensor("ohtd", [NJ * P, NCHP * P], bf16, kind="ExternalInput")
    waug1 = nc.dram_tensor("waug1", [P, 132], f32, kind="ExternalInput")
    waug2 = nc.dram_tensor("waug2", [P, 132], f32, kind="ExternalInput")
    waug3 = nc.dram_tensor("waug3", [P, P], f32, kind="ExternalInput")
    minv1d = nc.dram_tensor("minv1", [P, P], f32, kind="ExternalInput")
    minv2d = nc.dram_tensor("minv2", [P, P], f32, kind="ExternalInput")
    res3w = nc.dram_tensor("res3w", [P, HID], f32, kind="ExternalInput")
    wfc = nc.dram_tensor("wfc", [HID, N_CLS], f32, kind="ExternalInput")
    bias_d = [None] * 2
    bias_shapes = [(P, HID), (P, N_CLS)]
    for i, hb in enumerate(has_bias):
        if hb:
            bias_d[i] = nc.dram_tensor(f"bias{i}", list(bias_shapes[i]), f32,
                                       kind="ExternalInput")
    out_e = nc.dram_tensor("out", [shard, N_CLS], f32, kind="ExternalOutput")

    agA = [nc.dram_tensor(f"agA{l}", [BR, P], bf16, kind="Internal")
           for l in range(3)]
    agB = [nc.dram_tensor(f"agB{l}", [BR, P], bf16, kind="Internal")
           for l in range(3)]
    tables = [nc.dram_tensor(f"table{l}", [2 * n_cores * BR, P], bf16,
                             kind="Internal", addr_space="Shared")
              for l in range(3)]

    with tile.TileContext(nc) as tc:
        with (
            tc.tile_pool(name="const", bufs=1) as cpool,
            tc.tile_pool(name="big", bufs=1) as bigpool,
            tc.tile_pool(name="gth", bufs=2) as gpool,
            tc.tile_pool(name="oht", bufs=2) as opool,
            tc.tile_pool(name="work", bufs=3) as wpool,
            tc.tile_pool(name="wsm", bufs=3) as spool,
            tc.tile_pool(name="pagg", bufs=2, space="PSUM") as p_agg,
            tc.tile_pool(name="ptr", bufs=2, space="PSUM") as p_tr,
            tc.tile_pool(name="pdn", bufs=2, space="PSUM") as p_dn,
            tc.tile_pool(name="per", bufs=1, space="PSUM") as p_er,
            tc.tile_pool(name="prs", bufs=1, space="PSUM") as p_rs,
        ):
            ident = cpool.tile([P, P], f32)
            make_identity(nc, ident[:])
            iota_i = cpool.tile([P, P], i32)
            nc.gpsimd.iota(iota_i[:], pattern=[[1, P]], base=0, channel_multiplier=0)
            iota_bf = cpool.tile([P, P], bf16)
            nc.vector.tensor_copy(iota_bf[:], iota_i[:])

            w1_sb = cpool.tile([P, 132], f32)
            nc.sync.dma_start(w1_sb[:], waug1[:, :])
            w2_sb = cpool.tile([P, 132], f32)
            nc.sync.dma_start(w2_sb[:], waug2[:, :])
            w3_sb = cpool.tile([P, P], f32)
            nc.sync.dma_start(w3_sb[:], waug3[:, :])
            minv_sb = [cpool.tile([P, P], f32, name=f"minv{i}_sb") for i in range(2)]
            nc.sync.dma_start(minv_sb[0][:], minv1d[:, :])
            nc.sync.dma_start(minv_sb[1][:], minv2d[:, :])
            res3_sb = cpool.tile([P, HID], f32)
            nc.sync.dma_start(res3_sb[:], res3w[:, :])
            wfc_sb = cpool.tile([HID, N_CLS], f32)
            nc.sync.dma_start(wfc_sb[:], wfc[:, :])
            bias_sb = [None] * 2
            for i, d in enumerate(bias_d):
                if d is not None:
                    bias_sb[i] = cpool.tile(list(bias_shapes[i]), f32)
                    nc.sync.dma_start(bias_sb[i][:], d[:, :])

            h1T = bigpool.tile([P, SP], f32)
            h2T = bigpool.tile([P, SP], f32)
            er_sb = [bigpool.tile([P, T, 4], bf16, name=f"er{i}_sb") for i in range(3)]

            # one-time zeroing of the rotating gather buffers: all later
            # stale bytes are previously-gathered finite feats, masked by
            # the zero one-hot columns of padding slots.
            for _i in range(2):
                gz = gpool.tile([P, NCHP, P], bf16, tag="gsb")
                nc.vector.memset(gz[:], 0.0)

            def bcast_mid(ap, n):
                return bass.AP(ap.tensor, ap.offset, [ap.ap[0], [0, n], ap.ap[1]])

            def agin_write(li, t, fsb):
                if t < HB:
                    nc.sync.dma_start(agA[li][t * P:(t + 1) * P, :], fsb)
                else:
                    r = (t - HB) * P
                    nc.sync.dma_start(agB[li][r:r + P, :], fsb)

            def dense_tile(li, t, lhsT_ap):
                w_sb = (w1_sb, w2_sb, w3_sb)[li]
                ncols = (132, 132, P)[li]
                eroff, H = ((128, 4), (128, 4), (33, 1))[li]
                ps = p_dn.tile([P, ncols], f32, tag="ps_dense")
                nc.tensor.matmul(ps[:], lhsT=lhsT_ap, rhs=w_sb[:], start=True,
                                 stop=True)
                fsb = wpool.tile([P, P], bf16, tag="fsb")
                nc.scalar.activation(fsb[:], ps[:, :P], Act.Copy)
                agin_write(li, t, fsb[:])
                nc.scalar.activation(er_sb[li][:, t, :H], ps[:, eroff:eroff + H],
                                     Act.Copy)

            def cc_half(li, half):
                ag = (agA, agB)[half][li]
                o0 = half * n_cores * BR
                nc.gpsimd.collective_compute(
                    "AllGather", Alu.bypass, replica_groups=rg,
                    ins=[ag[:, :]],
                    outs=[tables[li][o0:o0 + n_cores * BR, :]])

            for t in range(T):
                lh = wpool.tile([P, P], f32, tag="xt_t")
                nc.sync.dma_start(lh[:], xT[:, t * P:(t + 1) * P])
                dense_tile(0, t, lh[:])
                if t == HB - 1:
                    cc_half(0, 0)
            cc_half(0, 1)

            def edge_pair(layer, j):
                li = layer - 1
                H = 4 if layer < 3 else 1
                FE = H * HID
                table = tables[li]
                act = layer < 3
                NCH = nchp[j]
                r0 = j * P

                meta = spool.tile([P, MW], i16, tag="meta")
                nc.sync.dma_start(meta[:], meta_d[r0:r0 + P, :])
                ohts = opool.tile([P, NCHP, P], bf16, tag="oht")
                nc.sync.dma_start(ohts[:, :NCH, :], ohtd[r0:r0 + P, :NCH * P])
                gsb = gpool.tile([P, NCHP, P], bf16, tag="gsb")
                for q in range(NQ):
                    for (cb, p16, nidx) in seg_plan[j][q]:
                        icols = (nidx + 15) // 16
                        nchq = (nidx + P - 1) // P
                        nc.gpsimd.dma_gather(
                            gsb[:, cb:cb + nchq, :],
                            table[q * QROWS:(q + 1) * QROWS, :],
                            meta[:, p16:p16 + icols],
                            num_idxs=nidx, num_idxs_reg=nidx, elem_size=P,
                            queue_num=q, single_packet=False,
                        )
                dofb = meta[:, ICW:ICW + NCH].bitcast(bf16)
                oh = wpool.tile([P, NCHP, P], bf16, tag="oh")
                nc.vector.tensor_tensor(
                    out=oh[:, :NCH, :], in0=bcast_mid(iota_bf[:, :], NCH),
                    in1=dofb.to_broadcast([P, NCH, P]), op=Alu.is_equal)
                # er expansion (per chunk, routed to its tile's er column)
                pse = p_er.tile([P, NCHP * 4], f32, tag="ps_er")
                for m in range(2):
                    t = 2 * j + m
                    for cidx in chunk_plan[j][m]:
                        nc.tensor.matmul(
                            pse[:, cidx * H:(cidx + 1) * H],
                            lhsT=ohts[:, cidx, :], rhs=er_sb[li][:, t, :H],
                            start=True, stop=True)
                el_ap = (gsb[:, :NCH, 0:P:HID] if act
                         else gsb[:, :NCH, HID:HID + 1])
                esb = spool.tile([P, NCHP, H], f32, tag="e")
                nc.vector.tensor_tensor(
                    out=esb[:, :NCH, :], in0=el_ap,
                    in1=pse[:, :NCH * H].rearrange("p (c h) -> p c h", h=H),
                    op=Alu.add)
                ex1 = spool.tile([P, NCHP, H], f32, tag="ex1")
                nc.scalar.activation(ex1[:, :NCH, :], esb[:, :NCH, :], Act.Exp)
                ex2 = spool.tile([P, NCHP, H], f32, tag="ex2")
                nc.scalar.activation(ex2[:, :NCH, :], esb[:, :NCH, :], Act.Exp,
                                     scale=NEG_SLOPE)
                ex = spool.tile([P, NCHP, H], f32, tag="ex")
                nc.vector.tensor_tensor(out=ex[:, :NCH, :], in0=ex1[:, :NCH, :],
                                        in1=ex2[:, :NCH, :], op=Alu.max)
                g = wpool.tile([P, NCHP, FE + H], bf16, tag="g")
                nc.vector.tensor_tensor(
                    out=g[:, :NCH, 0:FE].rearrange("p c (h d) -> p c h d", h=H),
                    in0=gsb[:, :NCH, 0:FE].rearrange("p c (h d) -> p c h d", h=H),
                    in1=ex[:, :NCH, :].to_broadcast([P, NCH, H, HID]),
                    op=Alu.mult)
                nc.scalar.activation(g[:, :NCH, FE:FE + H], ex[:, :NCH, :],
                                     Act.Copy)
                # aggregate per tile of the pair
                psa_m = []
                for m in range(2):
                    psa = p_agg.tile([P, FE + H], f32, tag="ps_agg")
                    cl = chunk_plan[j][m]
                    for ki, cidx in enumerate(cl):
                        nc.tensor.matmul(psa[:], lhsT=oh[:, cidx, :],
                                         rhs=g[:, cidx, :],
                                         start=(ki == 0), stop=(ki == len(cl) - 1))
                    psa_m.append(psa)
                for m in range(2):
                    t = 2 * j + m
                    r0t = t * P
                    psa = psa_m[m]
                    ssb = spool.tile([P, H], f32, tag="s")
                    nc.vector.tensor_scalar_max(ssb[:], psa[:, FE:FE + H], 1e-30)
                    rec = spool.tile([P, H], f32, tag="rec")
                    nc.vector.reciprocal(rec[:], ssb[:])
                    if act:
                        osb = wpool.tile([P, FE], f32, tag="osb")
                        nc.vector.tensor_tensor(
                            out=osb[:].rearrange("p (h d) -> p h d", h=H),
                            in0=psa[:, 0:FE].rearrange("p (h d) -> p h d", h=H),
                            in1=rec[:].to_broadcast([P, H, HID]), op=Alu.mult)
                        pst = p_tr.tile([P, P], f32, tag="ps_t")
                        nc.tensor.transpose(pst[:], osb[:], ident[:])
                        tsb = wpool.tile([P, P], f32, tag="tsb")
                        nc.scalar.activation(tsb[:], pst[:], Act.Copy)
                        psu = p_tr.tile([P, P], f32, tag="ps_t")
                        nc.tensor.matmul(psu[:], lhsT=minv_sb[li][:], rhs=tsb[:],
                                         start=True, stop=True)
                        if layer == 2:
                            srcb = wpool.tile([P, P], f32, tag="srcb")
                            nc.vector.tensor_tensor(out=srcb[:], in0=psu[:],
                                                    in1=h1T[:, r0t:r0t + P],
                                                    op=Alu.add)
                            src = srcb[:]
                        else:
                            src = psu[:]
                        rl = wpool.tile([P, P], f32, tag="rl")
                        nc.scalar.activation(rl[:], src, Act.Relu)
                        e3 = wpool.tile([P, P], f32, tag="e3")
                        nc.scalar.activation(e3[:], src, Act.Exp)
                        e4 = wpool.tile([P, P], f32, tag="e4")
                        nc.vector.tensor_scalar(e4[:], e3[:], 1.0, -1.0,
                                                Alu.min, Alu.add)
                        hdst = (h1T if layer == 1 else h2T)[:, r0t:r0t + P]
                        nc.vector.tensor_tensor(out=hdst, in0=rl[:], in1=e4[:],
                                                op=Alu.add)
                        dense_tile(li + 1, t, hdst)
                        if t == HB - 1:
                            cc_half(li + 1, 0)
                        elif t == T - 1:
                            cc_half(li + 1, 1)
                    else:
                        psr = p_rs.tile([P, HID], f32, tag="ps_res")
                        nc.tensor.matmul(psr[:], lhsT=h2T[:, r0t:r0t + P],
                                         rhs=res3_sb[:], start=True, stop=True)
                        osb = wpool.tile([P, HID], f32, tag="osb3")
                        nc.vector.tensor_tensor(
                            out=osb[:].rearrange("p (h d) -> p h d", h=1),
                            in0=psa[:, 0:HID].rearrange("p (h d) -> p h d", h=1),
                            in1=rec[:].to_broadcast([P, 1, HID]), op=Alu.mult)
                        nc.vector.tensor_tensor(out=osb[:], in0=osb[:],
                                                in1=psr[:], op=Alu.add)
                        if bias_sb[0] is not None:
                            nc.vector.tensor_tensor(out=osb[:], in0=osb[:],
                                                    in1=bias_sb[0][:, :],
                                                    op=Alu.add)
                        pst = p_tr.tile([P, P], f32, tag="ps_t")
                        nc.tensor.transpose(pst[:HID, :], osb[:], ident[:])
                        hts = spool.tile([HID, P], f32, tag="h3t")
                        nc.scalar.activation(hts[:], pst[:HID, :], Act.Copy)
                        psf = p_dn.tile([P, N_CLS], f32, tag="ps_dense")
                        nc.tensor.matmul(psf[:], lhsT=hts[:], rhs=wfc_sb[:],
                                         start=True, stop=True)
                        ofc = spool.tile([P, N_CLS], f32, tag="ofc")
                        nc.scalar.activation(ofc[:], psf[:], Act.Copy)
                        if bias_sb[1] is not None:
                            nc.vector.tensor_tensor(out=ofc[:], in0=ofc[:],
                                                    in1=bias_sb[1][:, :],
                                                    op=Alu.add)
                        rows = min(shard - r0t, P)
                        if rows > 0:
                            nc.sync.dma_start(out_e[r0t:r0t + rows, :],
                                              ofc[:rows, :])

            for layer in (1, 2, 3):
                for j in range(NJ):
                    edge_pair(layer, j)

    nc.compile()
    return nc


def _get_program(n_cores, plan, has_bias):
    key = (n_cores, plan["shard"], plan["NCHP"], plan["ICW"],
           tuple(plan["nchp"]), has_bias)
    if key not in _PROGRAM_CACHE:
        _PROGRAM_CACHE[key] = _build_program(n_cores, plan, has_bias)
    return _PROGRAM_CACHE[key]


def _make_in_maps(prep, inputs, has_bias, n_cores=N_CORES):
    W1 = np.asarray(inputs["W1"], np.float32).astype(np.float64)
    W2 = np.asarray(inputs["W2"], np.float32).astype(np.float64)
    W3 = np.asarray(inputs["W3"], np.float32).astype(np.float64)
    al1 = np.asarray(inputs["al1"], np.float32)
    al2 = np.asarray(inputs["al2"], np.float32)
    al3 = np.asarray(inputs["al3"], np.float32)
    M1, Mi1 = _head_basis(al1)
    M2, Mi2 = _head_basis(al2)
    waug1 = np.concatenate([W1 @ M1, _ar_proj(W1, np.asarray(inputs["ar1"]))],
                           axis=1).astype(np.float32)
    waug2 = np.concatenate([W2 @ M2, _ar_proj(W2, np.asarray(inputs["ar2"]))],
                           axis=1).astype(np.float32)
    al3p = _ar_proj(W3, al3)
    ar3p = _ar_proj(W3, np.asarray(inputs["ar3"]))
    waug3 = np.concatenate(
        [W3, al3p, ar3p, np.zeros((P, P - HID - 2), np.float64)],
        axis=1).astype(np.float32)

    biases = []
    shapes = [(P, HID), (P, N_CLS)]
    for i, nm in enumerate(("b3", "bfc")):
        b = np.asarray(inputs[nm], np.float32).reshape(1, -1)
        biases.append(np.ascontiguousarray(np.broadcast_to(b, shapes[i])))
    in_maps = []
    for c in range(n_cores):
        m = dict(
            xT=prep["xT_per_core"][c],
            meta=prep["meta_per_core"][c],
            ohtd=prep["oht_per_core"][c],
            waug1=waug1, waug2=waug2, waug3=waug3,
            minv1=Mi1.astype(np.float32), minv2=Mi2.astype(np.float32),
            res3w=np.asarray(inputs["res3"], np.float32),
            wfc=np.asarray(inputs["Wfc"], np.float32),
        )
        for i, hb in enumerate(has_bias):
            if hb:
                m[f"bias{i}"] = biases[i]
        in_maps.append(m)
    return in_maps


def run_gat(inputs, n_cores=N_CORES, trace=False):
    """Builds (cached), runs on hardware, returns (output, BassKernelResults)."""
    from concourse import bass_utils

    x, src, dst = inputs["x"], inputs["src"], inputs["dst"]
    prep = _prepare(x, src, dst, n_cores)
    assert not np.any(np.asarray(inputs["b1"])), "b1 bias not supported"
    assert not np.any(np.asarray(inputs["b2"])), "b2 bias not supported"
    has_bias = tuple(
        bool(np.any(np.asarray(inputs[nm]))) for nm in ("b3", "bfc"))
    nc = _get_program(n_cores, prep, has_bias)
    in_maps = _make_in_maps(prep, inputs, has_bias, n_cores)
    res = bass_utils.run_bass_kernel_spmd(
        nc, in_maps, core_ids=list(range(n_cores)), trace=trace)
    out = np.concatenate([r["out"] for r in res.results], axis=0)
    return out[: x.shape[0]].astype(np.float32), res


def kernel(**inputs):
    out, _ = run_gat(inputs)
    return out


# revision 9
# speedup vs baseline: 2.2445x; 1.1223x over previous
"""GAT (3-layer, DGL-style) forward pass on 8 Trainium2 NeuronCores.

Strategy (dst-node sharded, graph-parallel):
  - Nodes are partitioned into 8 contiguous shards (dst ownership); edges are
    grouped by owner(dst), dst-tile pair, table quarter (int16 gather range)
    and tile within the pair. Slots are chunk-major in a per-pair buffer, so
    one dma_gather per (pair, quarter) covers both tiles of the pair.
  - Per layer, each core computes feat_aug = h_shard @ [W@M | W@Ar] for its
    own shard. M is a per-head basis change whose first column is al, so
    el = feat'[h*32] comes back with the gather for free. The bf16 feat'
    table is exchanged via two half-shard AllGathers (the first overlaps the
    second half of the producing edge phase).
  - Edge phase per pair: 4 SWDGE gathers (queue per quarter) fetch src rows;
    ex = exp(leaky_relu(el+er)) is built as max(exp(e), exp(0.2e)); er is
    expanded edge-wise with a host-precomputed transposed one-hot matmul;
    one-hot matmuls accumulate sum(ex*feat') and sum(ex) per dst node in
    PSUM (softmax max-subtraction cancels; logits are O(1)). Gather tails
    are left stale (finite) and masked by the one-hots; the gather pool is
    zeroed once at startup.
  - Epilogue (layers 1-2): normalize, transpose, un-prime via an Minv
    matmul, residual + ELU in the transposed layout; the result directly
    feeds the next layer's dense matmul and stays in SBUF for residuals.
    Layer 3: projected residual and the classifier fused; rows DMA'd out.

All core-dependent information lives in per-core input tensors, so every core
runs an identical SPMD program.
"""

import sys

import numpy as np

for _p in ("/opt/trn_rl_repo",):
    if _p not in sys.path:
        sys.path.insert(0, _p)

import ml_dtypes

BF16 = ml_dtypes.bfloat16

P = 128
NEG_SLOPE = 0.2
HID = 32
N_CLS = 40
N_CORES = 8
NQ = 4
NI_HW_MAX = 1024

_PROGRAM_CACHE = {}


# ----------------------------------------------------------------------------
# Host-side preparation (index manipulation / sharding only)
# ----------------------------------------------------------------------------

def _head_basis(al):
    """Per-head basis M (first column = al_h) and its exact inverse."""
    H, D = al.shape
    M = np.zeros((H * D, H * D), np.float64)
    Minv = np.zeros((H * D, H * D), np.float64)
    for h in range(H):
        a = al[h].astype(np.float64)
        nrm2 = a @ a
        assert nrm2 > 1e-12
        Q, _ = np.linalg.qr(np.concatenate([a[:, None], np.eye(D)], axis=1))
        Mh = np.concatenate([a[:, None], Q[:, 1:D]], axis=1)
        Mih = np.concatenate([(a / nrm2)[None, :], Q[:, 1:D].T], axis=0)
        M[h * D:(h + 1) * D, h * D:(h + 1) * D] = Mh
        Minv[h * D:(h + 1) * D, h * D:(h + 1) * D] = Mih
    return M, Minv


def _ar_proj(W, ar):
    H, D = ar.shape
    A = np.zeros((H * D, H), np.float64)
    A[np.arange(H * D), np.arange(H * D) // D] = ar.reshape(-1).astype(np.float64)
    return W.astype(np.float64) @ A


def _prepare(x, src, dst, n_cores=N_CORES):
    n_nodes = x.shape[0]
    assert n_nodes % n_cores == 0
    shard = n_nodes // n_cores
    shard_pad = ((shard + P - 1) // P) * P
    T = shard_pad // P
    assert T % 2 == 0
    NJ = T // 2                 # tile pairs
    HB = T // 2                 # collective half boundary (tiles)
    BR = shard_pad // 2         # rows per half-block
    nrows = 2 * n_cores * BR
    assert nrows % NQ == 0
    QROWS = nrows // NQ
    assert QROWS <= 32767

    owner = dst // shard
    lt = dst - owner * shard
    tt = lt // P
    dofv = (lt % P).astype(np.int64)
    src_c = (src // shard).astype(np.int64)
    src_l = (src % shard).astype(np.int64)
    kh = src_l // BR
    srow = (kh * (n_cores * BR) + src_c * BR + (src_l - kh * BR)).astype(np.int64)
    quarter = srow // QROWS
    qidx = (srow % QROWS).astype(np.int16)

    jj = tt // 2
    mm = tt % 2
    group = ((owner * NJ + jj) * NQ + quarter) * 2 + mm
    n_groups = n_cores * NJ * NQ * 2
    counts = np.bincount(group, minlength=n_groups).reshape(n_cores, NJ, NQ, 2)
    ni = counts.max(axis=0)                      # [NJ, NQ, 2]
    ch = (ni + P - 1) // P                       # chunks
    # chunk base per (j, q, m) in the pair buffer; quarters outer, tile inner
    chq = ch.sum(axis=2)                         # [NJ, NQ]
    qbase = np.concatenate([np.zeros((NJ, 1), np.int64),
                            np.cumsum(chq, axis=1)[:, :NQ - 1]], axis=1)
    B = qbase[:, :, None] + np.concatenate(
        [np.zeros((NJ, NQ, 1), np.int64), ch[:, :, :1]], axis=2)   # [NJ,NQ,2]
    nchp = chq.sum(axis=1)                       # chunks per pair
    NCHP = int(nchp.max())

    # gather segments per (j, q): merged [num_idxs, chunk_base] or split
    # streams are padded to full 128-chunks (pad index 0 gathers a real,
    # finite row that the zero one-hot columns mask out), so every slot of
    # every chunk is always written — no stale-data reasoning needed.
    segs = [[None] * NQ for _ in range(NJ)]
    icol = np.zeros((NJ, NQ, 2), np.int64)       # stream 16-col base per seg
    icw_j = np.zeros(NJ, np.int64)
    for j in range(NJ):
        pos16 = 0
        for q in range(NQ):
            c0, c1 = int(ch[j, q, 0]), int(ch[j, q, 1])
            merged_n = (c0 + c1) * P
            if c0 and c1 and merged_n <= NI_HW_MAX:
                segs[j][q] = [(int(B[j, q, 0]), pos16, merged_n)]
                icol[j, q, 0] = pos16 * 16
                icol[j, q, 1] = pos16 * 16 + c0 * P
                pos16 += merged_n // 16
            else:
                ss = []
                for m, cm in ((0, c0), (1, c1)):
                    icol[j, q, m] = pos16 * 16
                    if cm:
                        ss.append((int(B[j, q, m]), pos16, cm * P))
                        pos16 += cm * P // 16
                segs[j][q] = ss
        icw_j[j] = pos16
    ICW = int(icw_j.max())
    MW = ICW + NCHP              # i16 meta columns: [gidx wrap | dof]

    order = np.argsort(group, kind="stable")
    g_sorted = group[order]
    starts = np.zeros(n_groups + 1, np.int64)
    np.cumsum(np.bincount(group, minlength=n_groups), out=starts[1:])
    pos = np.arange(len(order)) - starts[g_sorted]

    oc = g_sorted // (NJ * NQ * 2)
    rem = g_sorted % (NJ * NQ * 2)
    jc = rem // (NQ * 2)
    qc = (rem // 2) % NQ
    mc = rem % 2

    # slot in pair buffer (chunk-major)
    slot = B[jc, qc, mc] * P + pos
    pe = slot % P
    ce = slot // P
    dof16 = np.full((n_cores, NJ, P, NCHP),
                    np.float32(-1).astype(BF16).view(np.int16), np.int16)
    dof16[oc, jc, pe, ce] = dofv[order].astype(np.float32).astype(BF16).view(
        np.int16)
    dof_int = np.full((n_cores, NJ, P, NCHP), -1, np.int16)
    dof_int[oc, jc, pe, ce] = dofv[order].astype(np.int16)

    # gather index stream (16-wrapped)
    gpos = icol[jc, qc, mc] + pos
    gidx_all = np.zeros((n_cores, NJ, ICW * 16), np.int16)
    gidx_all[oc, jc, gpos] = qidx[order]
    gidx_w = gidx_all.reshape(n_cores, NJ, ICW, 16).transpose(0, 1, 3, 2)
    gidx_w = np.broadcast_to(gidx_w[:, :, None, :, :],
                             (n_cores, NJ, 8, 16, ICW))
    gidx_w = np.ascontiguousarray(gidx_w).reshape(n_cores, NJ * P, ICW)

    meta = np.zeros((n_cores, NJ, P, MW), np.int16)
    meta[:, :, :, :ICW] = gidx_w.reshape(n_cores, NJ, P, ICW)
    meta[:, :, :, ICW:] = dof16

    # transposed one-hot for er expansion
    i_ar = np.arange(P, dtype=np.int16)
    oht = (dof_int[:, :, None, :, :] == i_ar[None, None, :, None, None])
    oht = oht.transpose(0, 1, 2, 4, 3).astype(BF16)      # [C, NJ, i, c, p]
    oht = np.ascontiguousarray(oht.reshape(n_cores, NJ * P, NCHP * P))

    # chunk -> (tile-in-pair, is_first, is_last) per pair, in buffer order
    chunk_plan = []
    for j in range(NJ):
        plan = []
        for q in range(NQ):
            for m in range(2):
                for k in range(int(ch[j, q, m])):
                    plan.append((m, int(B[j, q, m]) + k))
        bym = {0: [], 1: []}
        for m, cidx in plan:
            bym[m].append(cidx)
        chunk_plan.append((bym[0], bym[1]))
    seg_plan = [[segs[j][q] for q in range(NQ)] for j in range(NJ)]

    xT_per_core = []
    for c in range(n_cores):
        xs = x[c * shard:(c + 1) * shard].astype(np.float32)
        if shard_pad != shard:
            xs = np.concatenate(
                [xs, np.zeros((shard_pad - shard, xs.shape[1]), np.float32)], 0)
        xT_per_core.append(np.ascontiguousarray(xs.T))

    return dict(
        shard=shard, shard_pad=shard_pad, T=T, NJ=NJ, HB=HB, BR=BR,
        QROWS=QROWS, NCHP=NCHP, ICW=ICW, MW=MW,
        nchp=nchp.tolist(),
        chunk_plan=chunk_plan, seg_plan=seg_plan,
        meta_per_core=[np.ascontiguousarray(
            meta[c].reshape(NJ * P, MW)) for c in range(n_cores)],
        oht_per_core=[np.ascontiguousarray(oht[c]) for c in range(n_cores)],
        xT_per_core=xT_per_core,
    )


# ----------------------------------------------------------------------------
# Device program
# ----------------------------------------------------------------------------

def _build_program(n_cores, plan, has_bias):
    from concourse import bacc, bass, tile
    import concourse.mybir as mybir
    from concourse.masks import make_identity

    dt = mybir.dt
    f32, bf16, i16, i32 = dt.float32, dt.bfloat16, dt.int16, dt.int32
    Alu = mybir.AluOpType
    Act = mybir.ActivationFunctionType

    shard, SP, T = plan["shard"], plan["shard_pad"], plan["T"]
    NJ, HB, BR, QROWS = plan["NJ"], plan["HB"], plan["BR"], plan["QROWS"]
    NCHP, ICW, MW = plan["NCHP"], plan["ICW"], plan["MW"]
    nchp = plan["nchp"]
    chunk_plan, seg_plan = plan["chunk_plan"], plan["seg_plan"]
    rg = [list(range(n_cores))]

    nc = bacc.Bacc("TRN2", target_bir_lowering=False, debug=False,
                   num_devices=n_cores, num_swdge_queues=4)

    xT = nc.dram_tensor("xT", [P, SP], f32, kind="ExternalInput")
    meta_d = nc.dram_tensor("meta", [NJ * P, MW], i16, kind="ExternalInput")
    ohtd = nc.dram_tensor("ohtd", [NJ * P, NCHP * P], bf16, kind="ExternalInput")
    waug1 = nc.dram_tensor("waug1", [P, 132], f32, kind="ExternalInput")
    waug2 = nc.dram_tensor("waug2", [P, 132], f32, kind="ExternalInput")
    waug3 = nc.dram_tensor("waug3", [P, P], f32, kind="ExternalInput")
    minv1d = nc.dram_tensor("minv1", [P, P], f32, kind="ExternalInput")
    minv2d = nc.dram_tensor("minv2", [P, P], f32, kind="ExternalInput")
    res3w = nc.dram_tensor("res3w", [P, HID], f32, kind="ExternalInput")
    wfc = nc.dram_tensor("wfc", [HID, N_CLS], f32, kind="ExternalInput")
    bias_d = [None] * 2
    bias_shapes = [(P, HID), (P, N_CLS)]
    for i, hb in enumerate(has_bias):
        if hb:
            bias_d[i] = nc.dram_tensor(f"bias{i}", list(bias_shapes[i]), f32,
                                       kind="ExternalInput")
    out_e = nc.dram_tensor("out", [shard, N_CLS], f32, kind="ExternalOutput")

    agA = [nc.dram_tensor(f"agA{l}", [BR, P], bf16, kind="Internal")
           for l in range(3)]
    agB = [nc.dram_tensor(f"agB{l}", [BR, P], bf16, kind="Internal")
           for l in range(3)]
    tables = [nc.dram_tensor(f"table{l}", [2 * n_cores * BR, P], bf16,
                             kind="Internal", addr_space="Shared")
              for l in range(3)]

    with tile.TileContext(nc) as tc:
        with (
            tc.tile_pool(name="const", bufs=1) as cpool,
            tc.tile_pool(name="big", bufs=1) as bigpool,
            tc.tile_pool(name="gth", bufs=3) as gpool,
            tc.tile_pool(name="oht", bufs=3) as opool,
            tc.tile_pool(name="work", bufs=3) as wpool,
            tc.tile_pool(name="wsm", bufs=3) as spool,
            tc.tile_pool(name="pagg", bufs=2, space="PSUM") as p_agg,
            tc.tile_pool(name="ptr", bufs=2, space="PSUM") as p_tr,
            tc.tile_pool(name="pdn", bufs=2, space="PSUM") as p_dn,
            tc.tile_pool(name="per", bufs=1, space="PSUM") as p_er,
            tc.tile_pool(name="prs", bufs=1, space="PSUM") as p_rs,
        ):
            ident = cpool.tile([P, P], f32)
            make_identity(nc, ident[:])
            iota_i = cpool.tile([P, P], i32)
            nc.gpsimd.iota(iota_i[:], pattern=[[1, P]], base=0, channel_multiplier=0)
            iota_bf = cpool.tile([P, P], bf16)
            nc.vector.tensor_copy(iota_bf[:], iota_i[:])

            w1_sb = cpool.tile([P, 132], f32)
            nc.sync.dma_start(w1_sb[:], waug1[:, :])
            w2_sb = cpool.tile([P, 132], f32)
            nc.sync.dma_start(w2_sb[:], waug2[:, :])
            w3_sb = cpool.tile([P, P], f32)
            nc.sync.dma_start(w3_sb[:], waug3[:, :])
            minv_sb = [cpool.tile([P, P], f32, name=f"minv{i}_sb") for i in range(2)]
            nc.sync.dma_start(minv_sb[0][:], minv1d[:, :])
            nc.sync.dma_start(minv_sb[1][:], minv2d[:, :])
            res3_sb = cpool.tile([P, HID], f32)
            nc.sync.dma_start(res3_sb[:], res3w[:, :])
            wfc_sb = cpool.tile([HID, N_CLS], f32)
            nc.sync.dma_start(wfc_sb[:], wfc[:, :])
            bias_sb = [None] * 2
            for i, d in enumerate(bias_d):
                if d is not None:
                    bias_sb[i] = cpool.tile(list(bias_shapes[i]), f32)
                    nc.sync.dma_start(bias_sb[i][:], d[:, :])

            h1T = bigpool.tile([P, SP], f32)
            h2T = bigpool.tile([P, SP], f32)
            er_sb = [bigpool.tile([P, T, 4], bf16, name=f"er{i}_sb") for i in range(3)]

            # one-time zeroing of the rotating gather buffers: all later
            # stale bytes are previously-gathered finite feats, masked by
            # the zero one-hot columns of padding slots.
            for _i in range(3):
                gz = gpool.tile([P, NCHP, P], bf16, tag="gsb")
                nc.vector.memset(gz[:], 0.0)

            def bcast_mid(ap, n):
                return bass.AP(ap.tensor, ap.offset, [ap.ap[0], [0, n], ap.ap[1]])

            def agin_write(li, t, fsb):
                if t < HB:
                    nc.sync.dma_start(agA[li][t * P:(t + 1) * P, :], fsb)
                else:
                    r = (t - HB) * P
                    nc.sync.dma_start(agB[li][r:r + P, :], fsb)

            def dense_tile(li, t, lhsT_ap):
                w_sb = (w1_sb, w2_sb, w3_sb)[li]
                ncols = (132, 132, P)[li]
                eroff, H = ((128, 4), (128, 4), (33, 1))[li]
                ps = p_dn.tile([P, ncols], f32, tag="ps_dense")
                nc.tensor.matmul(ps[:], lhsT=lhsT_ap, rhs=w_sb[:], start=True,
                                 stop=True)
                fsb = wpool.tile([P, P], bf16, tag="fsb")
                nc.scalar.activation(fsb[:], ps[:, :P], Act.Copy)
                agin_write(li, t, fsb[:])
                nc.scalar.activation(er_sb[li][:, t, :H], ps[:, eroff:eroff + H],
                                     Act.Copy)

            def cc_half(li, half):
                ag = (agA, agB)[half][li]
                o0 = half * n_cores * BR
                nc.gpsimd.collective_compute(
                    "AllGather", Alu.bypass, replica_groups=rg,
                    ins=[ag[:, :]],
                    outs=[tables[li][o0:o0 + n_cores * BR, :]])

            for t in range(T):
                lh = wpool.tile([P, P], f32, tag="xt_t")
                nc.sync.dma_start(lh[:], xT[:, t * P:(t + 1) * P])
                dense_tile(0, t, lh[:])
                if t == HB - 1:
                    cc_half(0, 0)
            cc_half(0, 1)

            def edge_pair(layer, j):
                li = layer - 1
                H = 4 if layer < 3 else 1
                FE = H * HID
                table = tables[li]
                act = layer < 3
                NCH = nchp[j]
                r0 = j * P

                meta = spool.tile([P, MW], i16, tag="meta")
                nc.sync.dma_start(meta[:], meta_d[r0:r0 + P, :])
                ohts = opool.tile([P, NCHP, P], bf16, tag="oht")
                nc.sync.dma_start(ohts[:, :NCH, :], ohtd[r0:r0 + P, :NCH * P])
                gsb = gpool.tile([P, NCHP, P], bf16, tag="gsb")
                for q in range(NQ):
                    for (cb, p16, nidx) in seg_plan[j][q]:
                        icols = (nidx + 15) // 16
                        nchq = (nidx + P - 1) // P
                        nc.gpsimd.dma_gather(
                            gsb[:, cb:cb + nchq, :],
                            table[q * QROWS:(q + 1) * QROWS, :],
                            meta[:, p16:p16 + icols],
                            num_idxs=nidx, num_idxs_reg=nidx, elem_size=P,
                            queue_num=q, single_packet=False,
                        )
                dofb = meta[:, ICW:ICW + NCH].bitcast(bf16)
                oh = wpool.tile([P, NCHP, P], bf16, tag="oh", bufs=2)
                nc.vector.tensor_tensor(
                    out=oh[:, :NCH, :], in0=bcast_mid(iota_bf[:, :], NCH),
                    in1=dofb.to_broadcast([P, NCH, P]), op=Alu.is_equal)
                # er expansion (per chunk, routed to its tile's er column)
                pse = p_er.tile([P, NCHP * 4], f32, tag="ps_er")
                for m in range(2):
                    t = 2 * j + m
                    for cidx in chunk_plan[j][m]:
                        nc.tensor.matmul(
                            pse[:, cidx * H:(cidx + 1) * H],
                            lhsT=ohts[:, cidx, :], rhs=er_sb[li][:, t, :H],
                            start=True, stop=True)
                el_ap = (gsb[:, :NCH, 0:P:HID] if act
                         else gsb[:, :NCH, HID:HID + 1])
                esb = spool.tile([P, NCHP, H], f32, tag="e")
                nc.vector.tensor_tensor(
                    out=esb[:, :NCH, :], in0=el_ap,
                    in1=pse[:, :NCH * H].rearrange("p (c h) -> p c h", h=H),
                    op=Alu.add)
                ex1 = spool.tile([P, NCHP, H], f32, tag="ex1")
                nc.scalar.activation(ex1[:, :NCH, :], esb[:, :NCH, :], Act.Exp)
                ex2 = spool.tile([P, NCHP, H], f32, tag="ex2")
                nc.scalar.activation(ex2[:, :NCH, :], esb[:, :NCH, :], Act.Exp,
                                     scale=NEG_SLOPE)
                ex = spool.tile([P, NCHP, H], f32, tag="ex")
                nc.vector.tensor_tensor(out=ex[:, :NCH, :], in0=ex1[:, :NCH, :],
                                        in1=ex2[:, :NCH, :], op=Alu.max)
                g = wpool.tile([P, NCHP, FE + H], bf16, tag="g", bufs=2)
                nc.vector.tensor_tensor(
                    out=g[:, :NCH, 0:FE].rearrange("p c (h d) -> p c h d", h=H),
                    in0=gsb[:, :NCH, 0:FE].rearrange("p c (h d) -> p c h d", h=H),
                    in1=ex[:, :NCH, :].to_broadcast([P, NCH, H, HID]),
                    op=Alu.mult)
                nc.scalar.activation(g[:, :NCH, FE:FE + H], ex[:, :NCH, :],
                                     Act.Copy)
                # aggregate per tile of the pair
                psa_m = []
                for m in range(2):
                    psa = p_agg.tile([P, FE + H], f32, tag="ps_agg")
                    cl = chunk_plan[j][m]
                    for ki, cidx in enumerate(cl):
                        nc.tensor.matmul(psa[:], lhsT=oh[:, cidx, :],
                                         rhs=g[:, cidx, :],
                                         start=(ki == 0), stop=(ki == len(cl) - 1))
                    psa_m.append(psa)
                for m in range(2):
                    t = 2 * j + m
                    r0t = t * P
                    psa = psa_m[m]
                    ssb = spool.tile([P, H], f32, tag="s")
                    nc.vector.tensor_scalar_max(ssb[:], psa[:, FE:FE + H], 1e-30)
                    rec = spool.tile([P, H], f32, tag="rec")
                    nc.vector.reciprocal(rec[:], ssb[:])
                    if act:
                        osb = wpool.tile([P, FE], f32, tag="osb")
                        nc.vector.tensor_tensor(
                            out=osb[:].rearrange("p (h d) -> p h d", h=H),
                            in0=psa[:, 0:FE].rearrange("p (h d) -> p h d", h=H),
                            in1=rec[:].to_broadcast([P, H, HID]), op=Alu.mult)
                        pst = p_tr.tile([P, P], f32, tag="ps_t")
                        nc.tensor.transpose(pst[:], osb[:], ident[:])
                        tsb = wpool.tile([P, P], f32, tag="tsb")
                        nc.scalar.activation(tsb[:], pst[:], Act.Copy)
                        psu = p_tr.tile([P, P], f32, tag="ps_t")
                        nc.tensor.matmul(psu[:], lhsT=minv_sb[li][:], rhs=tsb[:],
                                         start=True, stop=True)
                        if layer == 2:
                            srcb = wpool.tile([P, P], f32, tag="srcb")
                            nc.vector.tensor_tensor(out=srcb[:], in0=psu[:],
                                                    in1=h1T[:, r0t:r0t + P],
                                                    op=Alu.add)
                            src = srcb[:]
                        else:
                            src = psu[:]
                        rl = wpool.tile([P, P], f32, tag="rl")
                        nc.scalar.activation(rl[:], src, Act.Relu)
                        e3 = wpool.tile([P, P], f32, tag="e3")
                        nc.scalar.activation(e3[:], src, Act.Exp)
                        e4 = wpool.tile([P, P], f32, tag="e4")
                        nc.vector.tensor_scalar(e4[:], e3[:], 1.0, -1.0,
                                                Alu.min, Alu.add)
                        hdst = (h1T if layer == 1 else h2T)[:, r0t:r0t + P]
                        nc.vector.tensor_tensor(out=hdst, in0=rl[:], in1=e4[:],
                                                op=Alu.add)
                        dense_tile(li + 1, t, hdst)
                        if t == HB - 1:
                            cc_half(li + 1, 0)
                        elif t == T - 1:
                            cc_half(li + 1, 1)
                    else:
                        psr = p_rs.tile([P, HID], f32, tag="ps_res")
                        nc.tensor.matmul(psr[:], lhsT=h2T[:, r0t:r0t + P],
                                         rhs=res3_sb[:], start=True, stop=True)
                        osb = wpool.tile([P, HID], f32, tag="osb3")
                        nc.vector.tensor_tensor(
                            out=osb[:].rearrange("p (h d) -> p h d", h=1),
                            in0=psa[:, 0:HID].rearrange("p (h d) -> p h d", h=1),
                            in1=rec[:].to_broadcast([P, 1, HID]), op=Alu.mult)
                        nc.vector.tensor_tensor(out=osb[:], in0=osb[:],
                                                in1=psr[:], op=Alu.add)
                        if bias_sb[0] is not None:
                            nc.vector.tensor_tensor(out=osb[:], in0=osb[:],
                                                    in1=bias_sb[0][:, :],
                                                    op=Alu.add)
                        pst = p_tr.tile([P, P], f32, tag="ps_t")
                        nc.tensor.transpose(pst[:HID, :], osb[:], ident[:])
                        hts = spool.tile([HID, P], f32, tag="h3t")
                        nc.scalar.activation(hts[:], pst[:HID, :], Act.Copy)
                        psf = p_dn.tile([P, N_CLS], f32, tag="ps_dense")
                        nc.tensor.matmul(psf[:], lhsT=hts[:], rhs=wfc_sb[:],
                                         start=True, stop=True)
                        ofc = spool.tile([P, N_CLS], f32, tag="ofc")
                        nc.scalar.activation(ofc[:], psf[:], Act.Copy)
                        if bias_sb[1] is not None:
                            nc.vector.tensor_tensor(out=ofc[:], in0=ofc[:],
                                                    in1=bias_sb[1][:, :],
                                                    op=Alu.add)
                        rows = min(shard - r0t, P)
                        if rows > 0:
                            nc.sync.dma_start(out_e[r0t:r0t + rows, :],
                                              ofc[:rows, :])

            for layer in (1, 2, 3):
                for j in range(NJ):
                    edge_pair(layer, j)

    nc.compile()
    return nc


def _get_program(n_cores, plan, has_bias):
    key = (n_cores, plan["shard"], plan["NCHP"], plan["ICW"],
           tuple(plan["nchp"]), has_bias)
    if key not in _PROGRAM_CACHE:
        _PROGRAM_CACHE[key] = _build_program(n_cores, plan, has_bias)
    return _PROGRAM_CACHE[key]


def _make_in_maps(prep, inputs, has_bias, n_cores=N_CORES):
    W1 = np.asarray(inputs["W1"], np.float32).astype(np.float64)
    W2 = np.asarray(inputs["W2"], np.float32).astype(np.float64)
    W3 = np.asarray(inputs["W3"], np.float32).astype(np.float64)
    al1 = np.asarray(inputs["al1"], np.float32)
    al2 = np.asarray(inputs["al2"], np.float32)
    al3 = np.asarray(inputs["al3"], np.float32)
    M1, Mi1 = _head_basis(al1)
    M2, Mi2 = _head_basis(al2)
    waug1 = np.concatenate([W1 @ M1, _ar_proj(W1, np.asarray(inputs["ar1"]))],
                           axis=1).astype(np.float32)
    waug2 = np.concatenate([W2 @ M2, _ar_proj(W2, np.asarray(inputs["ar2"]))],
                           axis=1).astype(np.float32)
    al3p = _ar_proj(W3, al3)
    ar3p = _ar_proj(W3, np.asarray(inputs["ar3"]))
    waug3 = np.concatenate(
        [W3, al3p, ar3p, np.zeros((P, P - HID - 2), np.float64)],
        axis=1).astype(np.float32)

    biases = []
    shapes = [(P, HID), (P, N_CLS)]
    for i, nm in enumerate(("b3", "bfc")):
        b = np.asarray(inputs[nm], np.float32).reshape(1, -1)
        biases.append(np.ascontiguousarray(np.broadcast_to(b, shapes[i])))
    in_maps = []
    for c in range(n_cores):
        m = dict(
            xT=prep["xT_per_core"][c],
            meta=prep["meta_per_core"][c],
            ohtd=prep["oht_per_core"][c],
            waug1=waug1, waug2=waug2, waug3=waug3,
            minv1=Mi1.astype(np.float32), minv2=Mi2.astype(np.float32),
            res3w=np.asarray(inputs["res3"], np.float32),
            wfc=np.asarray(inputs["Wfc"], np.float32),
        )
        for i, hb in enumerate(has_bias):
            if hb:
                m[f"bias{i}"] = biases[i]
        in_maps.append(m)
    return in_maps


def run_gat(inputs, n_cores=N_CORES, trace=False):
    """Builds (cached), runs on hardware, returns (output, BassKernelResults)."""
    from concourse import bass_utils

    x, src, dst = inputs["x"], inputs["src"], inputs["dst"]
    prep = _prepare(x, src, dst, n_cores)
    assert not np.any(np.asarray(inputs["b1"])), "b1 bias not supported"
    assert not np.any(np.asarray(inputs["b2"])), "b2 bias not supported"
    has_bias = tuple(
        bool(np.any(np.asarray(inputs[nm]))) for nm in ("b3", "bfc"))
    nc = _get_program(n_cores, prep, has_bias)
    in_maps = _make_in_maps(prep, inputs, has_bias, n_cores)
    res = bass_utils.run_bass_kernel_spmd(
        nc, in_maps, core_ids=list(range(n_cores)), trace=trace)
    out = np.concatenate([r["out"] for r in res.results], axis=0)
    return out[: x.shape[0]].astype(np.float32), res


def kernel(**inputs):
    out, _ = run_gat(inputs)
    return out
